# Initial kernel scaffold
#
# Trainium2 Bass kernel for nn_CustomAttention (softmax1 attention-weight map).
#
# reference math:
#   Q = (query @ Wq.T + bq) / 1  -> heads [B, NQ, 16, 64]
#   K = (key   @ Wk.T + bk)
#   S_h = Q_h @ K_h.T / 8
#   softmax1: A_h = exp(S) / (1 + sum_k exp(S)),  a0_h = 1 / (1 + sum_k exp(S))
#   outputs: a0 = mean_h a0_h [B, NQ],  attn = mean_h A_h [B, NQ, NK]
#
# Sharding: 8 cores = 2 batches x 4 query-row blocks of 512. Each core computes
# its full 16-head softmax1 and head-mean locally (no collectives); host
# concatenates. The 1/8 score scale is folded into Wq/bq on the host.
#
# Per-core pipeline (all matmuls bf16, PSUM f32):
#   KpT = WkT-matmul(keyT)+bk   [1024, 2048]  (TensorE + VectorE bias/cast)
#   QpT = WqT-matmul(qT)+bq/8   [1024, 512]
#   per q-tile (128 rows), head h: S_h = QpT_h.T @ KpT_h (64-contraction,
#     packed 2 heads/row-halves via PE quadrant tiling), E_h = exp(S_h) on
#     ScalarE with accum_out giving Z_h = sum_k E_h.
#   per 4-head group: r = 1/(16*(1+Z)); diag(r_h) @ E_h accumulated in PSUM
#     across all 16 heads = head-mean attn; a0 = sum_h r_h.

import numpy as np
import ml_dtypes
from contextlib import ExitStack

EMBED = 1024
H = 16
D = 64
NK = 2048
NQL = 512            # query rows per core
QTILES = NQL // 128  # 4
CI = EMBED // 128    # 8 contraction chunks
PT = EMBED // 128    # 8 output-embed chunks
N_CORES = 8
HGRP = 4             # heads per r/diag group

_cache = {}


def _build_program():
    import concourse.bass as bass
    import concourse.mybir as mybir
    import concourse.tile as tile
    from concourse.masks import make_identity

    f32 = mybir.dt.float32
    bf16 = mybir.dt.bfloat16
    Alu = mybir.AluOpType
    Act = mybir.ActivationFunctionType

    nc = bass.Bass()
    qT = nc.declare_dram_parameter("qT", [128, CI, NQL], bf16)
    kT = nc.declare_dram_parameter("kT", [128, CI, NK], bf16)
    wqT = nc.declare_dram_parameter("wqT", [128, CI, EMBED], bf16)
    wkT = nc.declare_dram_parameter("wkT", [128, CI, EMBED], bf16)
    bqp = nc.declare_dram_parameter("bqp", [128, PT], f32)
    bkp = nc.declare_dram_parameter("bkp", [128, PT], f32)
    attn_d = nc.declare_dram_parameter("attn", [NQL, NK], f32, isOutput=True)
    a0_d = nc.declare_dram_parameter("a0", [QTILES, 128, 1], f32, isOutput=True)

    with ExitStack() as ctx:
        tc = ctx.enter_context(tile.TileContext(nc))
        const = ctx.enter_context(tc.tile_pool(name="const", bufs=1))
        proj = ctx.enter_context(tc.tile_pool(name="proj", bufs=1))
        epool = ctx.enter_context(tc.tile_pool(name="epool", bufs=10))
        spsum = ctx.enter_context(tc.tile_pool(name="spsum", bufs=2, space="PSUM"))
        apsum = ctx.enter_context(tc.tile_pool(name="apsum", bufs=1, space="PSUM"))
        dpool = ctx.enter_context(tc.tile_pool(name="dpool", bufs=8))
        zpool = ctx.enter_context(tc.tile_pool(name="zpool", bufs=2))
        opool = ctx.enter_context(tc.tile_pool(name="opool", bufs=2))

        k_sb = const.tile([128, CI, NK], bf16)
        nc.sync.dma_start(k_sb, kT)
        q_sb = const.tile([128, CI, NQL], bf16)
        nc.sync.dma_start(q_sb, qT)
        wq_sb = const.tile([128, CI, EMBED], bf16)
        nc.sync.dma_start(wq_sb, wqT)
        wk_sb = const.tile([128, CI, EMBED], bf16)
        nc.sync.dma_start(wk_sb, wkT)
        bq_sb = const.tile([128, PT], f32)
        nc.sync.dma_start(bq_sb, bqp)
        bk_sb = const.tile([128, PT], f32)
        nc.sync.dma_start(bk_sb, bkp)
        ident = const.tile([128, 128], bf16)
        make_identity(nc, ident)

        kp_sb = proj.tile([128, PT, NK], bf16)   # KpT: [e_out%128, e_out//128, k]
        qp_sb = proj.tile([128, PT, NQL], bf16)

        for pt in range(PT):
            for kh in range(2):
                ps = spsum.tile([128, 1024], f32, tag="sp", name="ps_k")
                for nn in range(2):
                    o = nn * 512
                    for ci in range(CI):
                        nc.tensor.matmul(
                            ps[:, o:o + 512],
                            lhsT=wk_sb[:, ci, pt * 128:(pt + 1) * 128],
                            rhs=k_sb[:, ci, kh * 1024 + o:kh * 1024 + o + 512],
                            start=(ci == 0), stop=(ci == CI - 1),
                        )
                nc.vector.tensor_scalar(
                    kp_sb[:, pt, kh * 1024:(kh + 1) * 1024], ps,
                    bk_sb[:, pt:pt + 1], None, Alu.add)
            psq = spsum.tile([128, 1024], f32, tag="sp", name="ps_q")
            for ci in range(CI):
                nc.tensor.matmul(
                    psq[:, :NQL],
                    lhsT=wq_sb[:, ci, pt * 128:(pt + 1) * 128],
                    rhs=q_sb[:, ci, :],
                    start=(ci == 0), stop=(ci == CI - 1),
                )
            nc.vector.tensor_scalar(
                qp_sb[:, pt, :], psq[:, :NQL],
                bq_sb[:, pt:pt + 1], None, Alu.add)

        for qt in range(QTILES):
            zq = zpool.tile([128, H, 2], f32, tag="zq", name="zq")
            rt = zpool.tile([128, H], f32, tag="rt", name="rt")
            A = apsum.tile([128, NK], f32, name="A")
            e_tiles = {}
            for g in range(H // HGRP):
                for h in range(g * HGRP, (g + 1) * HGRP):
                    pt = h // 2
                    base = (h % 2) * 64
                    et = epool.tile([128, NK], bf16, tag="E", name="et")
                    e_tiles[h] = et
                    for kh in range(2):
                        s = spsum.tile([128, 1024], f32, tag="sp", name="s")
                        for nn in range(2):
                            o = nn * 512
                            nc.tensor.matmul(
                                s[:, o:o + 512],
                                lhsT=qp_sb[base:base + 64, pt,
                                           qt * 128:(qt + 1) * 128],
                                rhs=kp_sb[base:base + 64, pt,
                                          kh * 1024 + o:kh * 1024 + o + 512],
                                start=True, stop=True,
                            )
                        nc.scalar.activation(
                            et[:, kh * 1024:(kh + 1) * 1024], s, Act.Exp,
                            accum_out=zq[:, h, kh:kh + 1],
                        )
                # r_h = (1/16) / (1 + Z_h) for this head group
                gs = slice(g * HGRP, (g + 1) * HGRP)
                rg = rt[:, gs]
                nc.vector.tensor_tensor(rg, zq[:, gs, 0], zq[:, gs, 1], Alu.add)
                nc.vector.tensor_scalar(rg, rg, 1.0, None, Alu.add)
                nc.vector.reciprocal(rg, rg)
                nc.vector.tensor_scalar(rg, rg, 1.0 / H, None, Alu.mult)
                for h in range(g * HGRP, (g + 1) * HGRP):
                    dg = dpool.tile([128, 128], bf16, tag="dg", name="dg")
                    nc.vector.tensor_scalar(dg, ident, rt[:, h:h + 1], None,
                                            Alu.mult)
                    for nn in range(4):
                        o = nn * 512
                        nc.tensor.matmul(
                            A[:, o:o + 512],
                            lhsT=dg,
                            rhs=e_tiles[h][:, o:o + 512],
                            start=(h == 0), stop=(h == H - 1),
                        )

            a0t = zpool.tile([128, 1], f32, tag="a0t", name="a0t")
            nc.vector.tensor_reduce(a0t, rt, axis=mybir.AxisListType.X,
                                    op=Alu.add)
            nc.sync.dma_start(a0_d[qt], a0t)

            Ao = opool.tile([128, NK], f32, tag="Ao", name="Ao")
            nc.vector.tensor_copy(Ao, A)
            nc.sync.dma_start(attn_d[qt * 128:(qt + 1) * 128, :], Ao)

    return nc


def _get_program():
    if "nc" not in _cache:
        _cache["nc"] = _build_program()
    return _cache["nc"]


def _swizzle(a):
    # [EMBED, F] -> [128, CI, F] with e = ci*128 + p on partitions
    f = a.shape[1]
    return np.ascontiguousarray(
        a.reshape(CI, 128, f).transpose(1, 0, 2))


def _bf16(a):
    return np.ascontiguousarray(a.astype(ml_dtypes.bfloat16))


def make_in_maps(query, key, Wq, bq, Wk, bk):
    query = np.asarray(query, np.float32)
    key = np.asarray(key, np.float32)
    Wq = np.asarray(Wq, np.float32)
    bq = np.asarray(bq, np.float32)
    Wk = np.asarray(Wk, np.float32)
    bk = np.asarray(bk, np.float32)

    scale = 1.0 / np.sqrt(D)
    wqT = _bf16(_swizzle((Wq * scale).T))     # lhsT[e_in, e_out], scaled
    wkT = _bf16(_swizzle(Wk.T))
    bqp = np.ascontiguousarray((bq * scale).reshape(PT, 128).T.astype(np.float32))
    bkp = np.ascontiguousarray(bk.reshape(PT, 128).T.astype(np.float32))

    in_maps = []
    for c in range(N_CORES):
        b = c // 4
        q0 = (c % 4) * NQL
        in_maps.append({
            "qT": _bf16(_swizzle(query[b, q0:q0 + NQL, :].T)),
            "kT": _bf16(_swizzle(key[b].T)),
            "wqT": wqT,
            "wkT": wkT,
            "bqp": bqp,
            "bkp": bkp,
        })
    return in_maps


def kernel(query, key, Wq, bq, Wk, bk):
    from concourse.bass_utils import run_bass_kernel_spmd

    nc = _get_program()
    in_maps = make_in_maps(query, key, Wq, bq, Wk, bk)
    res = run_bass_kernel_spmd(nc, in_maps, core_ids=list(range(N_CORES)))

    B, NQ = 2, 2048
    attn = np.empty((B, NQ, NK), np.float32)
    a0 = np.empty((B, NQ), np.float32)
    for c in range(N_CORES):
        b = c // 4
        q0 = (c % 4) * NQL
        r = res.results[c]
        attn[b, q0:q0 + NQL] = r["attn"]
        a0[b, q0:q0 + NQL] = r["a0"].reshape(NQL)
    return a0, attn


# revision 11
# speedup vs baseline: 5.0525x; 5.0525x over previous
# Trainium2 Bass kernel for nn_CustomAttention (softmax1 attention-weight map).
#
# reference math:
#   Q = (query @ Wq.T + bq) -> heads [B, NQ, 16, 64], scaled by 1/8
#   K = (key   @ Wk.T + bk)
#   S_h = Q_h @ K_h.T
#   softmax1: A_h = exp(S) / (1 + sum_k exp(S)),  a0_h = 1 / (1 + sum_k exp(S))
#   outputs: a0 = mean_h a0_h [B, NQ],  attn = mean_h A_h [B, NQ, NK]
#
# Sharding: 8 cores = 2 batches x 4 query-row blocks of 512. Each core computes
# its full 16-head softmax1 and head-mean locally (no collectives); host
# concatenates. The 1/8 score scale is folded into Wq/bq on the host.
#
# Per-core pipeline (all matmuls bf16, PSUM f32):
#   KpT = WkT-matmul(keyT)+bk   [1024, 2048]  (TensorE + VectorE bias/cast)
#   QpT = WqT-matmul(qT)+bq/8   [1024, 512]
#   per q-tile (128 rows), head h: S_h = QpT_h.T @ KpT_h (64-contraction,
#     packed 2 heads/row-halves via PE quadrant tiling), E_h = exp(S_h) on
#     ScalarE with accum_out giving Z_h = sum_k E_h.
#   per 4-head group: r = 1/(16*(1+Z)); diag(r_h) @ E_h accumulated in PSUM
#     across all 16 heads = head-mean attn; a0 = sum_h r_h.
#
# loop_n: wraps the whole per-core body in an on-device For_i loop (for
# benchmarking only -- amortizes the ~80ms axon dispatch overhead).

import numpy as np
import ml_dtypes
from contextlib import ExitStack

EMBED = 1024
H = 16
D = 64
NK = 2048
NQL = 512            # query rows per core
QTILES = NQL // 128  # 4
CI = EMBED // 128    # 8 contraction chunks
PT = EMBED // 128    # 8 output-embed chunks
N_CORES = 8
HGRP = 4             # heads per r/diag group

_cache = {}


def _build_program(loop_n=None):
    import concourse.bacc as bacc
    import concourse.mybir as mybir
    import concourse.tile as tile
    from concourse.masks import make_identity

    f32 = mybir.dt.float32
    bf16 = mybir.dt.bfloat16
    Alu = mybir.AluOpType
    Act = mybir.ActivationFunctionType
    ET = mybir.EngineType

    nc = bacc.Bacc()
    qT = nc.declare_dram_parameter("qT", [128, CI, NQL], bf16, isOutput=False)
    kT = nc.declare_dram_parameter("kT", [128, CI, NK], bf16, isOutput=False)
    wqT = nc.declare_dram_parameter("wqT", [128, CI, EMBED], bf16, isOutput=False)
    wkT = nc.declare_dram_parameter("wkT", [128, CI, EMBED], bf16, isOutput=False)
    bqp = nc.declare_dram_parameter("bqp", [128, PT], f32, isOutput=False)
    bkp = nc.declare_dram_parameter("bkp", [128, PT], f32, isOutput=False)
    attn_d = nc.declare_dram_parameter("attn", [NQL, NK], f32, isOutput=True)
    a0_d = nc.declare_dram_parameter("a0", [QTILES, 128, 1], f32, isOutput=True)

    with ExitStack() as ctx:
        tc = ctx.enter_context(tile.TileContext(nc))
        const = ctx.enter_context(tc.tile_pool(name="const", bufs=1))
        proj = ctx.enter_context(tc.tile_pool(name="proj", bufs=1))
        epool = ctx.enter_context(tc.tile_pool(name="epool", bufs=10))
        spsum = ctx.enter_context(tc.tile_pool(name="spsum", bufs=2, space="PSUM"))
        apsum = ctx.enter_context(tc.tile_pool(name="apsum", bufs=1, space="PSUM"))
        dpool = ctx.enter_context(tc.tile_pool(name="dpool", bufs=8))
        zpool = ctx.enter_context(tc.tile_pool(name="zpool", bufs=2))
        opool = ctx.enter_context(tc.tile_pool(name="opool", bufs=2))

        def emit():
            k_sb = const.tile([128, CI, NK], bf16, name="k_sb")
            nc.sync.dma_start(k_sb, kT[:])
            q_sb = const.tile([128, CI, NQL], bf16, name="q_sb")
            nc.sync.dma_start(q_sb, qT[:])
            wq_sb = const.tile([128, CI, EMBED], bf16, name="wq_sb")
            nc.sync.dma_start(wq_sb, wqT[:])
            wk_sb = const.tile([128, CI, EMBED], bf16, name="wk_sb")
            nc.sync.dma_start(wk_sb, wkT[:])
            bq_sb = const.tile([128, PT], f32, name="bq_sb")
            nc.sync.dma_start(bq_sb, bqp[:])
            bk_sb = const.tile([128, PT], f32, name="bk_sb")
            nc.sync.dma_start(bk_sb, bkp[:])
            ident = const.tile([128, 128], bf16, name="ident")
            make_identity(nc, ident)

            # KpT: [e_out%128, e_out//128, k]
            kp_sb = proj.tile([128, PT, NK], bf16, name="kp_sb")
            qp_sb = proj.tile([128, PT, NQL], bf16, name="qp_sb")

            for pt in range(PT):
                for kh in range(2):
                    ps = spsum.tile([128, 1024], f32, tag="sp", name="ps_k")
                    for nn in range(2):
                        o = nn * 512
                        for ci in range(CI):
                            nc.tensor.matmul(
                                ps[:, o:o + 512],
                                lhsT=wk_sb[:, ci, pt * 128:(pt + 1) * 128],
                                rhs=k_sb[:, ci, kh * 1024 + o:kh * 1024 + o + 512],
                                start=(ci == 0), stop=(ci == CI - 1),
                            )
                    nc.vector.tensor_tensor(
                        kp_sb[:, pt, kh * 1024:(kh + 1) * 1024], ps,
                        bk_sb[:, pt:pt + 1].to_broadcast((128, 1024)), Alu.add)
                psq = spsum.tile([128, 1024], f32, tag="sp", name="ps_q")
                for ci in range(CI):
                    nc.tensor.matmul(
                        psq[:, :NQL],
                        lhsT=wq_sb[:, ci, pt * 128:(pt + 1) * 128],
                        rhs=q_sb[:, ci, :],
                        start=(ci == 0), stop=(ci == CI - 1),
                    )
                nc.vector.tensor_tensor(
                    qp_sb[:, pt, :], psq[:, :NQL],
                    bq_sb[:, pt:pt + 1].to_broadcast((128, NQL)), Alu.add)

            for qt in range(QTILES):
                zq = zpool.tile([128, H, 2], f32, tag="zq", name="zq")
                rt = zpool.tile([128, H], f32, tag="rt", name="rt")
                A = apsum.tile([128, NK], f32, name="A")
                e_tiles = {}
                for g in range(H // HGRP):
                    for h in range(g * HGRP, (g + 1) * HGRP):
                        pt = h // 2
                        base = (h % 2) * 64
                        et = epool.tile([128, NK], bf16, tag="E", name="et")
                        e_tiles[h] = et
                        for kh in range(2):
                            s = spsum.tile([128, 1024], f32, tag="sp", name="s")
                            for nn in range(2):
                                o = nn * 512
                                nc.tensor.matmul(
                                    s[:, o:o + 512],
                                    lhsT=qp_sb[base:base + 64, pt,
                                               qt * 128:(qt + 1) * 128],
                                    rhs=kp_sb[base:base + 64, pt,
                                              kh * 1024 + o:kh * 1024 + o + 512],
                                    start=True, stop=True,
                                )
                            nc.scalar.activation(
                                et[:, kh * 1024:(kh + 1) * 1024], s, Act.Exp,
                                accum_out=zq[:, h, kh:kh + 1],
                            )
                    # r_h = (1/16) / (1 + Z_h) for this head group
                    gs = slice(g * HGRP, (g + 1) * HGRP)
                    rg = rt[:, gs]
                    nc.vector.tensor_tensor(rg, zq[:, gs, 0], zq[:, gs, 1],
                                            Alu.add)
                    nc.vector.tensor_scalar(rg, rg, 1.0, None, Alu.add)
                    nc.vector.reciprocal(rg, rg)
                    nc.vector.tensor_scalar(rg, rg, 1.0 / H, None, Alu.mult)
                    for h in range(g * HGRP, (g + 1) * HGRP):
                        dg = dpool.tile([128, 128], bf16, tag="dg", name="dg")
                        nc.vector.tensor_tensor(
                            dg, ident, rt[:, h:h + 1].to_broadcast((128, 128)),
                            Alu.mult)
                        for nn in range(4):
                            o = nn * 512
                            nc.tensor.matmul(
                                A[:, o:o + 512],
                                lhsT=dg,
                                rhs=e_tiles[h][:, o:o + 512],
                                start=(h == 0), stop=(h == H - 1),
                            )

                a0t = zpool.tile([128, 1], f32, tag="a0t", name="a0t")
                nc.vector.tensor_reduce(a0t, rt, axis=mybir.AxisListType.X,
                                        op=Alu.add)
                nc.sync.dma_start(a0_d[qt], a0t)

                Ao = opool.tile([128, NK], f32, tag="Ao", name="Ao")
                nc.vector.tensor_copy(Ao, A)
                nc.sync.dma_start(attn_d[qt * 128:(qt + 1) * 128, :], Ao)

        if loop_n is None:
            emit()
        else:
            with tc.For_i(0, loop_n, 1):
                emit()

    nc.compile()
    return nc


def _get_program(loop_n=None):
    key = ("nc", loop_n)
    if key not in _cache:
        _cache[key] = _build_program(loop_n)
    return _cache[key]


def _swizzle(a):
    # [EMBED, F] -> [128, CI, F] with e = ci*128 + p on partitions
    f = a.shape[1]
    return np.ascontiguousarray(
        a.reshape(CI, 128, f).transpose(1, 0, 2))


def _bf16(a):
    return np.ascontiguousarray(a.astype(ml_dtypes.bfloat16))


def make_in_maps(query, key, Wq, bq, Wk, bk):
    query = np.asarray(query, np.float32)
    key = np.asarray(key, np.float32)
    Wq = np.asarray(Wq, np.float32)
    bq = np.asarray(bq, np.float32)
    Wk = np.asarray(Wk, np.float32)
    bk = np.asarray(bk, np.float32)

    scale = 1.0 / np.sqrt(D)
    wqT = _bf16(_swizzle((Wq * scale).T))     # lhsT[e_in, e_out], scaled
    wkT = _bf16(_swizzle(Wk.T))
    bqp = np.ascontiguousarray((bq * scale).reshape(PT, 128).T.astype(np.float32))
    bkp = np.ascontiguousarray(bk.reshape(PT, 128).T.astype(np.float32))

    in_maps = []
    for c in range(N_CORES):
        b = c // 4
        q0 = (c % 4) * NQL
        in_maps.append({
            "qT": _bf16(_swizzle(query[b, q0:q0 + NQL, :].T)),
            "kT": _bf16(_swizzle(key[b].T)),
            "wqT": wqT,
            "wkT": wkT,
            "bqp": bqp,
            "bkp": bkp,
        })
    return in_maps


def kernel(query, key, Wq, bq, Wk, bk):
    from concourse.bass_utils import run_bass_kernel_spmd

    nc = _get_program()
    in_maps = make_in_maps(query, key, Wq, bq, Wk, bk)
    res = run_bass_kernel_spmd(nc, in_maps, core_ids=list(range(N_CORES)))

    B, NQ = 2, 2048
    attn = np.empty((B, NQ, NK), np.float32)
    a0 = np.empty((B, NQ), np.float32)
    for c in range(N_CORES):
        b = c // 4
        q0 = (c % 4) * NQL
        r = res.results[c]
        attn[b, q0:q0 + NQL] = r["attn"]
        a0[b, q0:q0 + NQL] = r["a0"].reshape(NQL)
    return a0, attn


# revision 28
# speedup vs baseline: 7.8762x; 1.5589x over previous
# Trainium2 Bass kernel for nn_CustomAttention (softmax1 attention-weight map).
#
# reference math:
#   Q = (query @ Wq.T + bq) -> heads [B, NQ, 16, 64], scaled by 1/8
#   K = (key   @ Wk.T + bk)
#   S_h = Q_h @ K_h.T
#   softmax1: A_h = exp(S) / (1 + sum_k exp(S)),  a0_h = 1 / (1 + sum_k exp(S))
#   outputs: a0 = mean_h a0_h [B, NQ],  attn = mean_h A_h [B, NQ, NK]
#
# Sharding: 8 cores = 2 batches x 4 query-row blocks of 512. Each core computes
# its full 16-head softmax1 and head-mean locally (no collectives); host
# concatenates. The 1/8 score scale is folded into Wq/bq on the host.
#
# Per-core pipeline (all matmuls bf16, PSUM f32):
#   KpT = WkT-matmul(keyT)+bk   [1024, 2048]  (TensorE + VectorE bias/cast)
#   QpT = WqT-matmul(qT)+bq/8   [1024, 512]
#   per q-tile (128 rows), head h: S_h = QpT_h.T @ KpT_h (64-contraction,
#     packed 2 heads/row-halves via PE quadrant tiling), E_h = exp(S_h) on
#     ScalarE with accum_out giving Z_h = sum_k E_h.
#   per 4-head group: r = 1/(16*(1+Z)); diag(r_h) @ E_h accumulated in PSUM
#     across all 16 heads = head-mean attn; a0 = sum_h r_h.
#
# loop_n: wraps the whole per-core body in an on-device For_i loop (for
# benchmarking only -- amortizes the ~80ms axon dispatch overhead).

import numpy as np
import ml_dtypes
from contextlib import ExitStack

EMBED = 1024
H = 16
D = 64
NK = 2048
NQL = 512            # query rows per core
QTILES = NQL // 128  # 4
CI = EMBED // 128    # 8 contraction chunks
PT = EMBED // 128    # 8 output-embed chunks
N_CORES = 8
HGRP = 4             # heads per r/diag group
KP_ALLGATHER = True  # shard K-projection across the 4-core batch group

_cache = {}


def _build_program(loop_n=None, emulate_cc=False):
    import concourse.bacc as bacc
    import concourse.mybir as mybir
    import concourse.tile as tile
    from concourse.masks import make_identity

    f32 = mybir.dt.float32
    bf16 = mybir.dt.bfloat16
    Alu = mybir.AluOpType
    Act = mybir.ActivationFunctionType
    ET = mybir.EngineType

    nc = bacc.Bacc()
    n_wk = 2 * 128 if KP_ALLGATHER else EMBED
    n_bk = 2 if KP_ALLGATHER else PT
    qT = nc.declare_dram_parameter("qT", [128, CI, NQL], bf16, isOutput=False)
    kT = nc.declare_dram_parameter("kT", [128, CI, NK], bf16, isOutput=False)
    wqT = nc.declare_dram_parameter("wqT", [128, CI, EMBED], bf16, isOutput=False)
    wkT = nc.declare_dram_parameter("wkT", [128, CI, n_wk], bf16, isOutput=False)
    bqp = nc.declare_dram_parameter("bqp", [128, PT], f32, isOutput=False)
    bkp = nc.declare_dram_parameter("bkp", [128, n_bk], f32, isOutput=False)
    attn_d = nc.declare_dram_parameter("attn", [NQL, NK], f32, isOutput=True)
    a0_d = nc.declare_dram_parameter("a0", [QTILES, 128, 1], f32, isOutput=True)
    if KP_ALLGATHER:
        # [kh, pt_local, p, k-half] and gathered [kh, pt, p, k-half]
        kp_loc = nc.dram_tensor("kp_loc", [2, 2, 128, 1024], bf16)
        kp_full = nc.dram_tensor("kp_full", [2, PT, 128, 1024], bf16)
        cc_groups = [[0, 1, 2, 3], [4, 5, 6, 7]]

    with ExitStack() as ctx:
        tc = ctx.enter_context(tile.TileContext(nc))
        const = ctx.enter_context(tc.tile_pool(name="const", bufs=1))
        proj = ctx.enter_context(tc.tile_pool(name="proj", bufs=1))
        epool = ctx.enter_context(tc.tile_pool(name="epool", bufs=12))
        spsum = ctx.enter_context(tc.tile_pool(name="spsum", bufs=2, space="PSUM"))
        apsum = ctx.enter_context(tc.tile_pool(name="apsum", bufs=1, space="PSUM"))
        dpool = ctx.enter_context(tc.tile_pool(name="dpool", bufs=8))
        zpool = ctx.enter_context(tc.tile_pool(name="zpool", bufs=3))
        opool = ctx.enter_context(tc.tile_pool(name="opool", bufs=2))

        def emit():
            k_sb = const.tile([128, CI, NK], bf16, name="k_sb")
            nc.sync.dma_start(k_sb, kT[:])
            q_sb = const.tile([128, CI, NQL], bf16, name="q_sb")
            nc.sync.dma_start(q_sb, qT[:])
            wq_sb = const.tile([128, CI, EMBED], bf16, name="wq_sb")
            nc.sync.dma_start(wq_sb, wqT[:])
            wk_sb = const.tile([128, CI, n_wk], bf16, name="wk_sb")
            nc.sync.dma_start(wk_sb, wkT[:])
            bq_sb = const.tile([128, PT], f32, name="bq_sb")
            nc.sync.dma_start(bq_sb, bqp[:])
            bk_sb = const.tile([128, n_bk], f32, name="bk_sb")
            nc.sync.dma_start(bk_sb, bkp[:])
            ident = const.tile([128, 128], bf16, name="ident")
            make_identity(nc, ident)

            # KpT: [e_out%128, e_out//128, k]
            kp_sb = proj.tile([128, PT, NK], bf16, name="kp_sb")
            qp_sb = proj.tile([128, PT, NQL], bf16, name="qp_sb")

            state = {}   # per-qt: zq, rt, A, e_tiles

            def begin_qt(qt):
                state[qt] = dict(
                    zq=zpool.tile([128, H, 2], f32, tag="zq", name="zq"),
                    rt=zpool.tile([128, H], f32, tag="rt", name="rt"),
                    A=apsum.tile([128, NK], f32, tag="A", name="A"),
                    e_tiles={},
                )

            def scores_exp(qt, h):
                st = state[qt]
                pt = h // 2
                base = (h % 2) * 64
                et = epool.tile([128, NK], bf16, tag="E", name="et")
                st["e_tiles"][h] = et
                for kh in range(2):
                    s = spsum.tile([128, 1024], f32, tag="sp", name="s")
                    for nn in range(2):
                        o = nn * 512
                        nc.tensor.matmul(
                            s[:, o:o + 512],
                            lhsT=qp_sb[base:base + 64, pt,
                                       qt * 128:(qt + 1) * 128],
                            rhs=kp_sb[base:base + 64, pt,
                                      kh * 1024 + o:kh * 1024 + o + 512],
                            start=True, stop=True,
                        )
                    nc.scalar.activation(
                        et[:, kh * 1024:(kh + 1) * 1024], s, Act.Exp,
                        accum_out=st["zq"][:, h, kh:kh + 1],
                    )

            pending = []   # (qt, h) diag work awaiting interleave

            def r_chain(qt, g):
                # r_h = (1/16) / (1 + Z_h) for head group g
                st = state[qt]
                zq, rt = st["zq"], st["rt"]
                gs = slice(g * HGRP, (g + 1) * HGRP)
                rg = rt[:, gs]
                nc.vector.tensor_tensor(rg, zq[:, gs, 0], zq[:, gs, 1],
                                        Alu.add)
                nc.vector.tensor_scalar(rg, rg, 1.0, None, Alu.add)
                nc.vector.reciprocal(rg, rg)
                nc.vector.tensor_scalar(rg, rg, 1.0 / H, None, Alu.mult)
                pending.extend((qt, h) for h in range(g * HGRP, (g + 1) * HGRP))

            def finish_qt(qt):
                st = state[qt]
                a0t = zpool.tile([128, 1], f32, tag="a0t", name="a0t")
                nc.vector.tensor_reduce(a0t, st["rt"],
                                        axis=mybir.AxisListType.X, op=Alu.add)
                nc.sync.dma_start(a0_d[qt], a0t)
                Ao = opool.tile([128, NK], f32, tag="Ao", name="Ao")
                nc.vector.tensor_copy(Ao, st["A"])
                nc.sync.dma_start(attn_d[qt * 128:(qt + 1) * 128, :], Ao)
                del state[qt]

            def drain_one_diag():
                # emit one pending head's diag(r_h) @ E_h accumulation; when a
                # qt's last head completes, evacuate its A
                if not pending:
                    return
                qt, h = pending.pop(0)
                st = state[qt]
                dg = dpool.tile([128, 128], bf16, tag="dg", name="dg")
                nc.vector.tensor_tensor(
                    dg, ident, st["rt"][:, h:h + 1].to_broadcast((128, 128)),
                    Alu.mult)
                for nn in range(4):
                    o = nn * 512
                    nc.tensor.matmul(
                        st["A"][:, o:o + 512],
                        lhsT=dg,
                        rhs=st["e_tiles"][h][:, o:o + 512],
                        start=(h == 0), stop=(h == H - 1),
                    )
                if h == H - 1:
                    finish_qt(qt)

            def kproj_tile(pt, kh, w_off, bias_col, out_ap):
                # one [128, 1024] tile of KpT = Wk-matmul(keyT) + bk
                ps = spsum.tile([128, 1024], f32, tag="sp", name="ps_k")
                for nn in range(2):
                    o = nn * 512
                    for ci in range(CI):
                        nc.tensor.matmul(
                            ps[:, o:o + 512],
                            lhsT=wk_sb[:, ci, w_off:w_off + 128],
                            rhs=k_sb[:, ci, kh * 1024 + o:kh * 1024 + o + 512],
                            start=(ci == 0), stop=(ci == CI - 1),
                        )
                nc.vector.tensor_tensor(
                    out_ap, ps,
                    bk_sb[:, bias_col:bias_col + 1].to_broadcast((128, 1024)),
                    Alu.add)

            def qproj(pt):
                psq = spsum.tile([128, 1024], f32, tag="sp", name="ps_q")
                for ci in range(CI):
                    nc.tensor.matmul(
                        psq[:, :NQL],
                        lhsT=wq_sb[:, ci, pt * 128:(pt + 1) * 128],
                        rhs=q_sb[:, ci, :],
                        start=(ci == 0), stop=(ci == CI - 1),
                    )
                nc.vector.tensor_tensor(
                    qp_sb[:, pt, :], psq[:, :NQL],
                    bq_sb[:, pt:pt + 1].to_broadcast((128, NQL)), Alu.add)

            begin_qt(0)
            if KP_ALLGATHER:
                # local K-projection (2 e_out tiles), gathered per k-half so
                # scores can start once the first half lands
                for kh in range(2):
                    for ptl in range(2):
                        kt = opool.tile([128, 1024], bf16, tag="kt", name="kt")
                        kproj_tile(None, kh, ptl * 128, ptl, kt)
                        nc.sync.dma_start(kp_loc[kh, ptl], kt)
                    if emulate_cc:
                        # timing twin: same data volume as the AllGather but
                        # via local DMA (collectives can't run under For_i)
                        for j in range(4):
                            nc.sync.dma_start(kp_full[kh, 2 * j:2 * j + 2],
                                              kp_loc[kh])
                    else:
                        nc.gpsimd.collective_compute(
                            "AllGather", Alu.bypass, replica_groups=cc_groups,
                            ins=[kp_loc[kh]], outs=[kp_full[kh]])
                    nc.sync.dma_start(
                        kp_sb[:, :, kh * 1024:(kh + 1) * 1024],
                        kp_full[kh].rearrange("t p k -> p t k"))
                for pt in range(PT):
                    qproj(pt)
                    scores_exp(0, 2 * pt)
                    drain_one_diag()
                    scores_exp(0, 2 * pt + 1)
                    drain_one_diag()
                    if pt % 2 == 1:
                        r_chain(0, pt // 2)
            else:
                # full local projections interleaved with qt0 attention: heads
                # (2pt, 2pt+1) depend only on projection pt
                for pt in range(PT):
                    for kh in range(2):
                        kproj_tile(pt, kh, pt * 128, pt,
                                   kp_sb[:, pt, kh * 1024:(kh + 1) * 1024])
                    qproj(pt)
                    scores_exp(0, 2 * pt)
                    drain_one_diag()
                    scores_exp(0, 2 * pt + 1)
                    drain_one_diag()
                    if pt % 2 == 1:
                        r_chain(0, pt // 2)

            # remaining q-tiles, diag work interleaved 1:1 with scores so PE
            # never bursts while ACT starves
            for qt in range(1, QTILES):
                begin_qt(qt)
                for g in range(H // HGRP):
                    for h in range(g * HGRP, (g + 1) * HGRP):
                        scores_exp(qt, h)
                        drain_one_diag()
                    r_chain(qt, g)
            while pending:
                drain_one_diag()

        if loop_n is None:
            emit()
        else:
            with tc.For_i(0, loop_n, 1, staggered_reset=True):
                emit()

    nc.compile()
    return nc


def _get_program(loop_n=None, emulate_cc=False):
    key = ("nc", loop_n, emulate_cc)
    if key not in _cache:
        _cache[key] = _build_program(loop_n, emulate_cc)
    return _cache[key]


def _swizzle(a):
    # [EMBED, F] -> [128, CI, F] with e = ci*128 + p on partitions
    f = a.shape[1]
    return np.ascontiguousarray(
        a.reshape(CI, 128, f).transpose(1, 0, 2))


def _bf16(a):
    return np.ascontiguousarray(a.astype(ml_dtypes.bfloat16))


def make_in_maps(query, key, Wq, bq, Wk, bk):
    query = np.asarray(query, np.float32)
    key = np.asarray(key, np.float32)
    Wq = np.asarray(Wq, np.float32)
    bq = np.asarray(bq, np.float32)
    Wk = np.asarray(Wk, np.float32)
    bk = np.asarray(bk, np.float32)

    scale = 1.0 / np.sqrt(D)
    wqT = _bf16(_swizzle((Wq * scale).T))     # lhsT[e_in, e_out], scaled
    wkT = _bf16(_swizzle(Wk.T))
    bqp = np.ascontiguousarray((bq * scale).reshape(PT, 128).T.astype(np.float32))
    bkp = np.ascontiguousarray(bk.reshape(PT, 128).T.astype(np.float32))

    in_maps = []
    for c in range(N_CORES):
        b = c // 4
        cp = c % 4
        q0 = cp * NQL
        if KP_ALLGATHER:
            wk_c = np.ascontiguousarray(wkT[:, :, cp * 256:(cp + 1) * 256])
            bk_c = np.ascontiguousarray(bkp[:, cp * 2:cp * 2 + 2])
        else:
            wk_c, bk_c = wkT, bkp
        in_maps.append({
            "qT": _bf16(_swizzle(query[b, q0:q0 + NQL, :].T)),
            "kT": _bf16(_swizzle(key[b].T)),
            "wqT": wqT,
            "wkT": wk_c,
            "bqp": bqp,
            "bkp": bk_c,
        })
    return in_maps


def kernel(query, key, Wq, bq, Wk, bk):
    from concourse.bass_utils import run_bass_kernel_spmd

    nc = _get_program()
    in_maps = make_in_maps(query, key, Wq, bq, Wk, bk)
    res = run_bass_kernel_spmd(nc, in_maps, core_ids=list(range(N_CORES)))

    B, NQ = 2, 2048
    attn = np.empty((B, NQ, NK), np.float32)
    a0 = np.empty((B, NQ), np.float32)
    for c in range(N_CORES):
        b = c // 4
        q0 = (c % 4) * NQL
        r = res.results[c]
        attn[b, q0:q0 + NQL] = r["attn"]
        a0[b, q0:q0 + NQL] = r["a0"].reshape(NQL)
    return a0, attn
